# revision 40
# baseline (speedup 1.0000x reference)
"""Trainium2 Bass kernel for nn_CWAUCHLoss (pairwise AUC hinge + class-weighted CE).

Math: with s = sigmoid(output[:, 0]), lab = labels[:, 0], the O(B^2) pairwise
penalty collapses algebraically (LAMB == 2):

  sum_{i in N, j in P} (1 - (s_j - s_i))^2
    = n_pos * sum_N s^2 + 2 * (sum_N s)(sum_P (1-s)) + n_neg * sum_P (1-s)^2

so the whole loss reduces to 7 masked sums over B elements:
  r0 = sum lab          r1 = sum s       r2 = sum s^2
  r3 = sum lab*s        r4 = sum lab*s^2
  q1 = sum ln(1+e^-s)   q2 = sum lab*ln(1+e^-s)
using ln(t) = -ln(1+e^-s) and ln(1-t) = ln(t) - s for t = sigmoid(s).

On-chip (one NeuronCore, batch spread over 128 partitions x 64 lanes):
  - sigmoid is computed as 1/(1+exp(-x)) so ALL transcendentals (Exp, Ln)
    live in one ACT table set (a Sigmoid->Ln sequence would pay a ~2.7us
    mid-kernel table switch); a dummy Exp prewarms the table during the
    input DMA, and a post-compile pass retargets the auto-inserted table
    load to the combined natural_log_exp_and_others set.
  - per-partition sums land in a [128, 8] stats tile via activation/STT
    accumulators; a matmul against the tile's own 1/128 constant column
    reduces across partitions; a second matmul against a constant 8x18
    matrix forms every linear combination; a 9-product bilinear form +
    grouped reduce + reciprocal yields [cls, penalty] in a few tiny DVE ops.
  - raw Bass (nc.Block) with per-engine counter semaphores: TRN2 engines
    are deep-pipelined with no scoreboard, so every RAW dependency (same-
    engine included) is sequenced through semaphores; skipping Tile's
    entry/exit barriers saves ~0.4us on a ~8.5us kernel.
"""

import numpy as np

B = 8192
P = 128
N = B // P  # 64 elements per partition

_nc_cache = None


def _wmat() -> np.ndarray:
    # Rows index the partition-reduced stats rc = [r0,r1,r2,r3,r4,1,q1,q2]/128.
    # Cols 0-8 build vector A, cols 9-17 build vector B; elementwise A*B then
    # group-sum by 3 gives [penalty numerator, penalty denominator, fpcls].
    W = np.zeros((8, 18), dtype=np.float64)
    Bf = float(B)
    W[0, 0] = 1.0                                     # A0 = r0
    W[1, 1] = 2.0
    W[3, 1] = -2.0                                    # A1 = 2(r1-r3)
    W[5, 2] = Bf
    W[0, 2] = -1.0                                    # A2 = B-r0
    W[0, 3] = 2.0                                     # A3 = 2 r0
    W[5, 6] = 1.0 / Bf                                # A6 = 1/B
    W[0, 7] = 1.0 / (Bf * Bf)                         # A7 = r0/B^2
    W[2, 9] = 1.0
    W[4, 9] = -1.0                                    # B0 = r2-r4
    W[0, 10] = 1.0
    W[3, 10] = -1.0                                   # B1 = r0-r3
    W[0, 11] = 1.0
    W[3, 11] = -2.0
    W[4, 11] = 1.0                                    # B2 = r0-2r3+r4
    W[5, 12] = Bf
    W[0, 12] = -1.0                                   # B3 = B-r0
    W[7, 15] = 1.0                                    # B6 = q2
    W[6, 16] = 1.0
    W[1, 16] = 1.0
    W[7, 16] = -2.0
    W[3, 16] = -1.0                                   # B7 = q1+r1-2q2-r3
    # rc carries true_sums/128 (the reduce matmul weights by the 1/128 const
    # column), so scale every coefficient by 128 to compensate.
    return np.ascontiguousarray(W * P, dtype=np.float32)


def build_nc():
    from contextlib import ExitStack

    import concourse.bacc as bacc
    import concourse.mybir as mybir

    f32 = mybir.dt.float32
    AF = mybir.ActivationFunctionType
    ALU = mybir.AluOpType
    AX = mybir.AxisListType

    nc = bacc.Bacc(None, target_bir_lowering=False, debug=False)
    x_d = nc.dram_tensor("output", [B, 2], f32, kind="ExternalInput")
    l_d = nc.dram_tensor("labels", [B, 1], f32, kind="ExternalInput")
    w_d = nc.dram_tensor("wmat", [8, 18], f32, kind="ExternalInput")
    o_d = nc.dram_tensor("out", [1, 2], f32, kind="ExternalOutput")

    with ExitStack() as ctx:
        e = ctx.enter_context
        xt = e(nc.sbuf_tensor([P, N, 2], f32))
        lt = e(nc.sbuf_tensor([P, N], f32))
        wt = e(nc.sbuf_tensor([8, 18], f32))
        e1 = e(nc.sbuf_tensor([P, N], f32))
        p1 = e(nc.sbuf_tensor([P, N], f32))
        s = e(nc.sbuf_tensor([P, N], f32))
        ls = e(nc.sbuf_tensor([P, N], f32))
        e2 = e(nc.sbuf_tensor([P, N], f32))
        lnw = e(nc.sbuf_tensor([P, N], f32))
        scr2 = e(nc.sbuf_tensor([P, N], f32))
        scr4 = e(nc.sbuf_tensor([P, N], f32))
        scrq = e(nc.sbuf_tensor([P, N], f32))
        ST = e(nc.sbuf_tensor([P, 8], f32))
        warm = e(nc.sbuf_tensor([1, 2], f32))
        bias01 = e(nc.sbuf_tensor([P, 2], f32))
        rcs = e(nc.sbuf_tensor([8, 1], f32))
        LCs = e(nc.sbuf_tensor([1, 18], f32))
        PPt = e(nc.sbuf_tensor([1, 9], f32))
        Ft = e(nc.sbuf_tensor([1, 4], f32))
        rd = e(nc.sbuf_tensor([1, 1], f32))
        G = e(nc.sbuf_tensor([1, 2], f32))
        psA = e(nc.psum_tensor([8, 1], f32))
        psB = e(nc.psum_tensor([1, 18], f32))
        d_x = e(nc.semaphore("d_x"))
        d_l = e(nc.semaphore("d_l"))
        d_w = e(nc.semaphore("d_w"))
        d_o = e(nc.semaphore("d_o"))
        ACTc = e(nc.semaphore("ACTc"))
        DVEc = e(nc.semaphore("DVEc"))
        PEc = e(nc.semaphore("PEc"))
        block = e(nc.Block())

        @block.sync
        def _(sync):
            # x first: it gates the whole compute chain (HWDGE descriptor
            # generation is a shared serial unit, ~625ns per dma_start).
            sync.dma_start(
                xt[:], x_d.ap().rearrange("(p n) c -> p n c", p=P)
            ).then_inc(d_x, 16)
            sync.dma_start(
                lt[:], l_d.ap().rearrange("(p n) c -> p (n c)", p=P)
            ).then_inc(d_l, 16)
            sync.wait_ge(DVEc, 19)  # G written
            sync.dma_start(o_d.ap(), G[:]).then_inc(d_o, 16)
            sync.wait_ge(d_o, 16)

        @block.gpsimd
        def _(gpsimd):
            # wmat is needed late (second matmul); SWDGE generation on the
            # otherwise-idle Pool engine runs parallel to the HWDGE unit.
            gpsimd.dma_start(wt[:], w_d.ap()).then_inc(d_w, 16)

        @block.scalar
        def _(scalar):
            scalar.wait_ge(DVEc, 3)  # bias01 + warm tile memsets
            # prewarm: pulls the exp/ln table set during the input DMA
            scalar.activation(
                warm[:], warm[:], AF.Exp, bias=bias01[0:1, 0:1]
            ).then_inc(ACTc, 1)  # 1
            scalar.wait_ge(d_x, 16)
            scalar.activation(
                e1[:], xt[:, :, 0], AF.Exp, scale=-1.0, bias=bias01[:, 0:1]
            ).then_inc(ACTc, 1)  # 2
            scalar.activation(
                e2[:], s[:], AF.Exp, scale=-1.0, bias=bias01[:, 0:1]
            ).then_inc(ACTc, 1)._wait_ge(DVEc, 7)  # 3
            # ln(1+e2): the +1 rides the Ln op's bias input; accum -> q1
            scalar.activation(
                lnw[:], e2[:], AF.Ln, bias=bias01[:, 1:2],
                accum_out=ST[:, 6:7],
            ).then_inc(ACTc, 1)._wait_ge(ACTc, 3)  # 4

        @block.vector
        def _(vector):
            # dep-free preamble memsets (compute path => inc-by-1 legal;
            # gpsimd memsets with inc-1 sems crash the device)
            vector.memset(bias01[:, 0:1], 0.0).then_inc(DVEc, 1)   # 1
            vector.memset(bias01[:, 1:2], 1.0).then_inc(DVEc, 1)   # 2
            vector.memset(warm[:], 1.0).then_inc(DVEc, 1)          # 3
            vector.memset(Ft[:, 3:4], 0.0).then_inc(DVEc, 1)       # 4
            vector.memset(ST[:, 5:6], 1.0 / P).then_inc(DVEc, 1)   # 5
            # s = sigmoid(x0) = 1/(1+e1); reciprocal on DVE is IEEE-exact
            vector.tensor_scalar_add(
                p1[:], e1[:], 1.0
            ).then_inc(DVEc, 1)._wait_ge(ACTc, 2)  # 6
            vector.reciprocal(s[:], p1[:]).then_inc(DVEc, 1)._wait_ge(DVEc, 6)  # 7
            # per-partition stats (fill DVE idle time under the ACT chain)
            vector.tensor_reduce(
                ST[:, 1:2], s[:], axis=AX.X, op=ALU.add
            ).then_inc(DVEc, 1)._wait_ge(DVEc, 7)  # 8
            vector.wait_ge(d_l, 16)
            vector.scalar_tensor_tensor(
                out=ls[:], in0=lt[:], scalar=1.0, in1=s[:],
                op0=ALU.mult, op1=ALU.mult, accum_out=ST[:, 3:4],
            ).then_inc(DVEc, 1)  # 9
            vector.scalar_tensor_tensor(
                out=scr2[:], in0=s[:], scalar=1.0, in1=s[:],
                op0=ALU.mult, op1=ALU.mult, accum_out=ST[:, 2:3],
            ).then_inc(DVEc, 1)  # 10
            vector.scalar_tensor_tensor(
                out=scr4[:], in0=ls[:], scalar=1.0, in1=ls[:],
                op0=ALU.mult, op1=ALU.mult, accum_out=ST[:, 4:5],
            ).then_inc(DVEc, 1)._wait_ge(DVEc, 9)  # 11
            vector.tensor_reduce(
                ST[:, 0:1], lt[:], axis=AX.X, op=ALU.add
            ).then_inc(DVEc, 1)  # 12
            vector.scalar_tensor_tensor(
                out=scrq[:], in0=lt[:], scalar=1.0, in1=lnw[:],
                op0=ALU.mult, op1=ALU.mult, accum_out=ST[:, 7:8],
            ).then_inc(DVEc, 1)._wait_ge(ACTc, 4)  # 13
            # tail: PSUM staging copies, bilinear products, grouped sums,
            # penalty = num/den, cls = fpcls + penalty
            vector.tensor_copy(rcs[:], psA[:]).then_inc(DVEc, 1)._wait_ge(PEc, 1)  # 14
            vector.tensor_copy(LCs[:], psB[:]).then_inc(DVEc, 1)._wait_ge(PEc, 2)  # 15
            vector.tensor_tensor(
                PPt[:], LCs[0:1, 0:9], LCs[0:1, 9:18], op=ALU.mult
            ).then_inc(DVEc, 1)._wait_ge(DVEc, 15)  # 16
            vector.tensor_reduce(
                Ft[:, 0:3],
                PPt[:].rearrange("p (g k) -> p g k", k=3),
                axis=AX.X,
                op=ALU.add,
            ).then_inc(DVEc, 1)._wait_ge(DVEc, 16)  # 17
            vector.reciprocal(rd[:], Ft[:, 1:2]).then_inc(DVEc, 1)._wait_ge(DVEc, 17)  # 18
            # G = [num/den + fpcls, num/den + 0] = [cls, penalty]
            vector.scalar_tensor_tensor(
                out=G[:],
                in0=Ft[:, 0:1].broadcast_to([1, 2]),
                scalar=rd[0:1, 0:1],
                in1=Ft[:, 2:4],
                op0=ALU.mult,
                op1=ALU.add,
            ).then_inc(DVEc, 1)._wait_ge(DVEc, 18)  # 19

        @block.tensor
        def _(tensor):
            tensor.wait_ge(ACTc, 4)   # lnw accum (q1)
            # cross-partition reduce: rc = ST^T @ (1/128 column)
            tensor.matmul(
                psA[:], ST[:, 0:8], ST[:, 5:6]
            ).then_inc(PEc, 1)._wait_ge(DVEc, 13)
            tensor.wait_ge(d_w, 16)   # wt
            # all linear combos: LC = rc^T @ W
            tensor.matmul(
                psB[:], rcs[:], wt[:]
            ).then_inc(PEc, 1)._wait_ge(DVEc, 14)

    nc.compile()

    # Table-load surgery: the greedy chooser assigns set 0 (exp_and_others)
    # to the Exp ops and then pays a second mid-chain ~1.3us load of set 5
    # (natural_log) before Ln.  Set 6 (natural_log_exp_and_others) contains
    # BOTH, so retarget the first load and drop the rest (they carry no
    # semaphore waits/updates).
    _COMBINED_EXP_LN_SET = 6
    for blk in nc.main_func.blocks:
        loads = [
            i for i in blk.instructions
            if isinstance(i, mybir.InstLoadActFuncSet)
        ]
        if not loads:
            continue
        assert all(not i.has_wait() and not i.has_update() for i in loads)
        loads[0].act_func_set_id = _COMBINED_EXP_LN_SET
        drop = {id(i) for i in loads[1:]}
        kept = [i for i in blk.instructions if id(i) not in drop]
        del blk.instructions[:]
        blk.instructions.extend(kept)

    # Drop Bass.__init__'s unconditional const-AP memsets (f32 0/1, bf16 1,
    # u8 127): nothing in this kernel reads them (biases come from bias01),
    # and they sit at the head of the Pool stream delaying startup.
    import json as _json

    for blk in nc.main_func.blocks:
        kept = []
        for i in blk.instructions:
            if isinstance(i, mybir.InstMemset) and not i.has_wait() and not i.has_update():
                j = _json.loads(mybir.instruction_to_pretty_json_string(i))
                memref = j.get("outs", [{}])[0].get("memref", "")
                if isinstance(memref, str) and memref.startswith("const-"):
                    continue
            kept.append(i)
        if len(kept) != len(blk.instructions):
            del blk.instructions[:]
            blk.instructions.extend(kept)
    return nc


def _in_map(output: np.ndarray, labels: np.ndarray) -> dict:
    return {
        "output": np.ascontiguousarray(output, dtype=np.float32),
        "labels": np.ascontiguousarray(labels, dtype=np.float32),
        "wmat": _wmat(),
    }


def kernel(output: np.ndarray, labels: np.ndarray) -> np.ndarray:
    global _nc_cache
    from concourse.bass_utils import run_bass_kernel_spmd

    if _nc_cache is None:
        _nc_cache = build_nc()
    res = run_bass_kernel_spmd(_nc_cache, [_in_map(output, labels)], core_ids=[0])
    g = res.results[0]["out"]
    return np.asarray(g, dtype=np.float32).reshape(2).copy()
